# revision 1
# baseline (speedup 1.0000x reference)
"""DiT block Bass kernel for 8 TRN2 NeuronCores.

Core i -> (b = i//4, g = i%4): batch item b; head group 4g..4g+3; token
quarter [512g, 512g+512) of batch b.  Activations are hidden-major
("transposed", [hidden_chunk=128, tokens]) throughout; PE transposes at
entry (x) and exit (out).  Collectives: AllGather(4) for mod + h,
AllToAll(4) for ctx.  Matmuls bf16 with f32 PSUM accumulate; softmax is
computed without max-subtraction (scores are provably small) with the
relative bias applied multiplicatively post-exp from a host-precomputed
diagonal-shifted exp(bias) table.
"""
import contextlib
import time
import numpy as np
import ml_dtypes
import jax
from jax.sharding import Mesh, PartitionSpec
from jax.experimental.shard_map import shard_map

import concourse.bass as bass
import concourse.mybir as mybir
import concourse.tile as tile
from concourse import bacc
from concourse.bass2jax import _bass_exec_p, install_neuronx_cc_hook, partition_id_tensor

F32 = mybir.dt.float32
BF16 = mybir.dt.bfloat16
AF = mybir.ActivationFunctionType
OP = mybir.AluOpType
ts = bass.ts

B, N, HID = 2, 2048, 1024
NH, HD = 16, 64
MLPH = 4 * HID
NB, MAXD = 32, 128
P = 128
TT = 512
KC = HID // P          # 8
NBLK = N // P          # 16
EB_A = 1920
EB_J = 3968
RG4 = [[0, 1, 2, 3], [4, 5, 6, 7]]


# ---------------------------------------------------------------- host prep
def rel_bucket_np(d):
    nb = NB // 2
    buckets = np.where(d > 0, nb, 0).astype(np.int64)
    rp = np.abs(d)
    max_exact = nb // 2
    is_small = rp < max_exact
    log_ratio = np.log(np.maximum(rp, 1).astype(np.float32) / np.float32(max_exact))
    rpl = max_exact + (
        log_ratio / np.float32(np.log(MAXD / max_exact)) * (nb - max_exact)
    ).astype(np.int32)
    rpl = np.minimum(rpl, nb - 1)
    return buckets + np.where(is_small, rp, rpl)


def make_eb_tables(rel_table):
    d = np.arange(-(N - 1), N)
    buck = rel_bucket_np(d)
    p = np.arange(P)[:, None]
    j = np.arange(EB_J)[None, :]
    dd = p + EB_A - j
    valid = (dd >= -(N - 1)) & (dd <= N - 1)
    idx = np.clip(dd + (N - 1), 0, 2 * N - 2)
    ebs = np.zeros((NH, P, EB_J), dtype=np.float32)
    for h in range(NH):
        bvec = rel_table[buck, h].astype(np.float32)
        tab = np.exp(bvec)[idx]
        tab[~valid] = 1.0
        ebs[h] = tab
    return ebs.astype(ml_dtypes.bfloat16)


def make_in_maps(inputs):
    x = np.asarray(inputs["x"], np.float32)
    c = np.asarray(inputs["c"], np.float32)
    w_ada = np.asarray(inputs["w_ada"], np.float32)
    b_ada = np.asarray(inputs["b_ada"], np.float32)
    w_qkv = np.asarray(inputs["w_qkv"], np.float32)
    b_qkv = np.asarray(inputs["b_qkv"], np.float32)
    w_out = np.asarray(inputs["w_out"], np.float32)
    b_out = np.asarray(inputs["b_out"], np.float32)
    rel_table = np.asarray(inputs["rel_table"], np.float32)
    w_mlp1 = np.asarray(inputs["w_mlp1"], np.float32)
    b_mlp1 = np.asarray(inputs["b_mlp1"], np.float32)
    w_mlp2 = np.asarray(inputs["w_mlp2"], np.float32)
    b_mlp2 = np.asarray(inputs["b_mlp2"], np.float32)

    eb_all = make_eb_tables(rel_table)
    ident = np.eye(P, dtype=np.float32)
    ones_col = np.ones((P, 1), np.float32)
    ones_row = np.ones((1, P), np.float32)

    maps = []
    for i in range(8):
        b, g = divmod(i, 4)
        qs, ks, vs = 256 * g, HID + 256 * g, 2 * HID + 256 * g
        w_qkv_s = np.concatenate(
            [w_qkv[:, qs:qs + 256], w_qkv[:, ks:ks + 256], w_qkv[:, vs:vs + 256]], 1)
        b_qk = np.concatenate([b_qkv[qs:qs + 256], b_qkv[ks:ks + 256]])
        bv = b_qkv[vs:vs + 256]
        maps.append({
            "x_own": np.ascontiguousarray(x[b, 512 * g:512 * (g + 1), :]),
            "c_own": np.ascontiguousarray(c[b][:, None]),
            "w_ada_s": np.ascontiguousarray(
                w_ada[:, 1536 * g:1536 * (g + 1)].reshape(KC, P, 12, P)
                .transpose(2, 1, 0, 3)),
            "b_ada_s": np.ascontiguousarray(
                b_ada[1536 * g:1536 * (g + 1)].reshape(12, P).T),
            "w_qk_r": np.ascontiguousarray(
                w_qkv_s[:, :512].reshape(KC, P, 4, P).transpose(2, 1, 0, 3)),
            "w_v_r": np.ascontiguousarray(
                w_qkv_s[:, 512:].reshape(KC, P, 256).transpose(1, 0, 2)),
            "b_qk_s": np.ascontiguousarray(b_qk.reshape(4, P).T),
            "b_v_bcast": np.ascontiguousarray(
                np.broadcast_to(bv[None, :], (P, 256)).astype(ml_dtypes.bfloat16)),
            "w_out_s": np.ascontiguousarray(
                w_out[256 * g:256 * (g + 1), :].reshape(2, P, HID)
                .transpose(1, 0, 2)),
            "b_out_r": np.ascontiguousarray(b_out.reshape(KC, P).T),
            "w_mlp1": np.ascontiguousarray(
                w_mlp1.reshape(KC, P, MLPH // P, P).transpose(2, 1, 0, 3)),
            "b_mlp1_r": np.ascontiguousarray(b_mlp1.reshape(MLPH // P, P).T),
            "w_mlp2": np.ascontiguousarray(
                w_mlp2.reshape(2, 16, P, KC, P).transpose(3, 0, 2, 1, 4)),
            "b_mlp2_r": np.ascontiguousarray(b_mlp2.reshape(KC, P).T),
            "eb": np.ascontiguousarray(eb_all[4 * g:4 * g + 4]),
            "ident": ident,
            "ones_col": ones_col,
            "ones_row": ones_row,
        })
    return maps


def assemble_output(results):
    out = np.zeros((B, N, HID), np.float32)
    for i in range(8):
        b, g = divmod(i, 4)
        out[b, 512 * g:512 * (g + 1), :] = results[i]["out"]
    return out


# ---------------------------------------------------------------- builder
def build_kernel(sim=False):
    nc = bacc.Bacc("TRN2", target_bir_lowering=False, debug=False, num_devices=8)

    din = lambda nm, sh, dt=F32: nc.dram_tensor(nm, sh, dt, kind="ExternalInput")
    x_own = din("x_own", [TT, HID])
    c_own = din("c_own", [HID, 1])
    w_ada_s = din("w_ada_s", [12, P, KC, P])
    b_ada_s = din("b_ada_s", [P, 12])
    w_qk_r = din("w_qk_r", [4, P, KC, P])
    w_v_r = din("w_v_r", [P, KC, 256])
    b_qk_s = din("b_qk_s", [P, 4])
    b_v_bcast = din("b_v_bcast", [P, 256], BF16)
    w_out_s = din("w_out_s", [P, 2, HID])
    b_out_r = din("b_out_r", [P, KC])
    w_mlp1 = din("w_mlp1", [MLPH // P, P, KC, P])
    b_mlp1_r = din("b_mlp1_r", [P, MLPH // P])
    w_mlp2 = din("w_mlp2", [KC, 2, P, 16, P])
    b_mlp2_r = din("b_mlp2_r", [P, KC])
    eb_in = din("eb", [4, P, EB_J], BF16)
    ident_in = din("ident", [P, P])
    ones_col_in = din("ones_col", [P, 1])
    ones_row_in = din("ones_row", [1, P])

    out_t = nc.dram_tensor("out", [TT, HID], F32, kind="ExternalOutput")

    with tile.TileContext(nc) as tc, contextlib.ExitStack() as ctx:
        const = ctx.enter_context(tc.tile_pool(name="const", bufs=1))
        pers = ctx.enter_context(tc.tile_pool(name="pers", bufs=1))
        big = ctx.enter_context(tc.tile_pool(name="big", bufs=1))
        work = ctx.enter_context(tc.tile_pool(name="work", bufs=3))
        wst = ctx.enter_context(tc.tile_pool(name="wst", bufs=2))
        dram = ctx.enter_context(tc.tile_pool(name="dram", bufs=1, space="DRAM"))
        ebp = ctx.enter_context(tc.tile_pool(name="ebp", bufs=2))
        ps_acc = ctx.enter_context(tc.tile_pool(name="ps_acc", bufs=4, space="PSUM"))
        ps_bc = ctx.enter_context(tc.tile_pool(name="ps_bc", bufs=2, space="PSUM"))
        ps_ctx = ctx.enter_context(tc.tile_pool(name="ps_ctx", bufs=2, space="PSUM"))

        # ---------------- constants
        ident = const.tile([P, P], F32)
        nc.sync.dma_start(ident[:], ident_in.ap())
        ones_col = const.tile([P, 1], F32)
        nc.sync.dma_start(ones_col[:], ones_col_in.ap())
        ones_row = const.tile([1, P], F32)
        nc.sync.dma_start(ones_row[:], ones_row_in.ap())
        b_qk_sb = const.tile([P, 4], F32)
        nc.sync.dma_start(b_qk_sb[:], b_qk_s.ap())
        b_v_sb = const.tile([P, 256], BF16)
        nc.sync.dma_start(b_v_sb[:], b_v_bcast.ap())
        b_out_sb = const.tile([P, KC], F32)
        nc.sync.dma_start(b_out_sb[:], b_out_r.ap())
        b_mlp1_sb = const.tile([P, MLPH // P], F32)
        nc.sync.dma_start(b_mlp1_sb[:], b_mlp1_r.ap())
        b_mlp2_sb = const.tile([P, KC], F32)
        nc.sync.dma_start(b_mlp2_sb[:], b_mlp2_r.ap())
        b_ada_sb = const.tile([P, 12], F32)
        nc.sync.dma_start(b_ada_sb[:], b_ada_s.ap())
        eps_sb = const.tile([1, 1], F32)
        nc.vector.memset(eps_sb[:], 1e-6)

        # ---------------- phase 0: mod shard (this core: w_ada cols 1536g..)
        cT_sb = pers.tile([P, KC], F32)
        nc.sync.dma_start(cT_sb[:], c_own.ap().rearrange("(c p) o -> p (c o)", p=P))
        silu_sb = pers.tile([P, KC], F32)
        nc.scalar.activation(silu_sb[:], cT_sb[:], AF.Silu)
        mod_sh_sb = pers.tile([P, 12], F32)
        for mu in range(12):
            wa = wst.tile([P, KC, P], F32, tag="wf")
            nc.sync.dma_start(wa[:], w_ada_s.ap()[mu])
            mps = ps_acc.tile([P, 1], F32, tag="acc")
            for kc in range(KC):
                nc.tensor.matmul(mps[:], wa[:, kc, :], silu_sb[:, kc:kc + 1],
                                 start=(kc == 0), stop=(kc == KC - 1))
            nc.vector.tensor_scalar_add(
                mod_sh_sb[:, mu:mu + 1], mps[:], b_ada_sb[:, mu:mu + 1])
        mod_bounce_in = dram.tile([P, 12], F32)
        nc.sync.dma_start(mod_bounce_in[:], mod_sh_sb[:])
        mod_bounce_out = dram.tile([4 * P, 12], F32)
        if sim:
            nc.sync.dma_start(mod_bounce_out[:][0:P, :], mod_bounce_in[:])
        else:
            nc.gpsimd.collective_compute(
                "AllGather", OP.bypass, replica_groups=RG4,
                ins=[mod_bounce_in.opt()], outs=[mod_bounce_out.opt()])
        mod_sb = pers.tile([P, 4, 12], F32)
        nc.sync.dma_start(
            mod_sb[:], mod_bounce_out[:].rearrange("(g p) j -> p g j", p=P))

        def mod_chunk(vec_idx, kc):
            gc = 8 * vec_idx + kc
            return mod_sb[:, gc // 12, gc % 12:gc % 12 + 1]

        sc1p_msa = pers.tile([P, KC], F32)
        sc1p_mlp = pers.tile([P, KC], F32)
        for kc in range(KC):
            nc.vector.tensor_scalar_add(sc1p_msa[:, kc:kc + 1], mod_chunk(1, kc), 1.0)
            nc.vector.tensor_scalar_add(sc1p_mlp[:, kc:kc + 1], mod_chunk(4, kc), 1.0)

        # ---------------- phase 1: xT via PE transpose
        xT = pers.tile([P, KC, TT], F32)
        for r in range(TT // P):
            x_sb = work.tile([P, HID], F32, tag="xrow", bufs=3)
            nc.sync.dma_start(x_sb[:], x_own.ap()[ts(r, P), :])
            for kc in range(KC):
                tps = ps_acc.tile([P, P], F32, tag="acc")
                nc.tensor.transpose(tps[:], x_sb[:, ts(kc, P)], ident[:])
                nc.vector.tensor_copy(xT[:, kc, ts(r, P)], tps[:])

        def ln_stats(src, tag):
            sum_ps = ps_acc.tile([1, TT], F32, tag="acc")
            for kc in range(KC):
                nc.tensor.matmul(sum_ps[:], ones_col[:], src[:, kc, :],
                                 start=(kc == 0), stop=(kc == KC - 1))
            sumsq_ps = ps_acc.tile([1, TT], F32, tag="acc")
            for kc in range(KC):
                sq = work.tile([P, TT], F32, tag="wf32", bufs=5)
                nc.scalar.activation(sq[:], src[:, kc, :], AF.Square)
                nc.tensor.matmul(sumsq_ps[:], ones_col[:], sq[:],
                                 start=(kc == 0), stop=(kc == KC - 1))
            m_row = work.tile([1, TT], F32, tag="rowtmp", bufs=4)
            nc.vector.tensor_scalar_mul(m_row[:], sum_ps[:], 1.0 / HID)
            msq = work.tile([1, TT], F32, tag="rowtmp", bufs=4)
            nc.vector.tensor_tensor(msq[:], m_row[:], m_row[:], op=OP.mult)
            var_row = work.tile([1, TT], F32, tag="rowtmp", bufs=4)
            nc.vector.scalar_tensor_tensor(
                var_row[:], sumsq_ps[:], 1.0 / HID, msq[:],
                op0=OP.mult, op1=OP.subtract)
            sd_row = work.tile([1, TT], F32, tag="rowtmp", bufs=4)
            nc.scalar.activation(sd_row[:], var_row[:], AF.Sqrt, bias=eps_sb[:])
            r_row = work.tile([1, TT], F32, tag="rowtmp", bufs=4)
            nc.vector.reciprocal(r_row[:], sd_row[:])
            m_bc = ps_bc.tile([P, TT], F32, tag="bc")
            nc.tensor.matmul(m_bc[:], ones_row[:], m_row[:], start=True, stop=True)
            r_bc = ps_bc.tile([P, TT], F32, tag="bc")
            nc.tensor.matmul(r_bc[:], ones_row[:], r_row[:], start=True, stop=True)
            return m_bc, r_bc

        # ---------------- phase 2: hT own + AllGather
        m_bc, r_bc = ln_stats(xT, "ln1")
        hT_own = big.tile([P, KC, TT], BF16, tag="slot32")
        for kc in range(KC):
            t0 = work.tile([P, TT], F32, tag="wf32", bufs=5)
            nc.vector.tensor_sub(t0[:], xT[:, kc, :], m_bc[:])
            t1 = work.tile([P, TT], F32, tag="wf32", bufs=5)
            nc.vector.tensor_tensor(t1[:], t0[:], r_bc[:], op=OP.mult)
            nc.vector.tensor_scalar(
                hT_own[:, kc, :], t1[:], sc1p_msa[:, kc:kc + 1], mod_chunk(0, kc),
                op0=OP.mult, op1=OP.add)
        h_bounce_in_a = dram.tile([HID // 2, TT], BF16)
        h_bounce_in_b = dram.tile([HID // 2, TT], BF16)
        nc.sync.dma_start(
            h_bounce_in_a[:].rearrange("(c p) t -> p c t", p=P), hT_own[:, 0:4, :])
        nc.sync.dma_start(
            h_bounce_in_b[:].rearrange("(c p) t -> p c t", p=P), hT_own[:, 4:8, :])
        h_bounce_out_a = dram.tile([2 * HID, TT], BF16)
        h_bounce_out_b = dram.tile([2 * HID, TT], BF16)
        if sim:
            nc.sync.dma_start(h_bounce_out_a[:][0:HID // 2, :], h_bounce_in_a[:])
            nc.sync.dma_start(h_bounce_out_b[:][0:HID // 2, :], h_bounce_in_b[:])
        else:
            nc.gpsimd.collective_compute(
                "AllGather", OP.bypass, replica_groups=RG4,
                ins=[h_bounce_in_a.opt()], outs=[h_bounce_out_a.opt()])
            nc.gpsimd.collective_compute(
                "AllGather", OP.bypass, replica_groups=RG4,
                ins=[h_bounce_in_b.opt()], outs=[h_bounce_out_b.opt()])
        hT_full = big.tile([P, 32, TT], BF16, tag="slot32")
        for jq in range(4):
            nc.sync.dma_start(
                hT_full[:, KC * jq:KC * jq + 4, :],
                h_bounce_out_a[:][ts(jq, HID // 2), :].rearrange("(c p) t -> p c t", p=P))
            nc.sync.dma_start(
                hT_full[:, KC * jq + 4:KC * jq + 8, :],
                h_bounce_out_b[:][ts(jq, HID // 2), :].rearrange("(c p) t -> p c t", p=P))

        # ---------------- phase 3: qkv
        qT = pers.tile([P, 2, N], BF16)
        kT = pers.tile([P, 2, N], BF16)
        v_aug = pers.tile([P, NBLK, 260], BF16)
        nc.vector.memset(
            v_aug[:].rearrange("p b (h e) -> p b h e", h=4)[:, :, :, 64:65], 1.0)

        wvf = wst.tile([P, KC, 256], F32, tag="wf")
        nc.sync.dma_start(wvf[:], w_v_r.ap())
        wvb = wst.tile([P, KC, 256], BF16, tag="wb")
        nc.scalar.activation(wvb[:], wvf[:], AF.Copy)
        for blk in range(NBLK):
            ps = ps_acc.tile([P, 256], F32, tag="acc")
            for kc in range(KC):
                nc.tensor.matmul(
                    ps[:], hT_full[:, 8 * (blk // 4) + kc, ts(blk % 4, P)],
                    wvb[:, kc, :], start=(kc == 0), stop=(kc == KC - 1))
            vtmp = work.tile([P, 256], BF16, tag="wbf", bufs=6)
            nc.vector.tensor_copy(vtmp[:], ps[:])
            nc.vector.tensor_add(
                v_aug[:, blk, :].rearrange("p (h e) -> p h e", h=4)[:, :, 0:64],
                vtmp[:].rearrange("p (h e) -> p h e", h=4), b_v_sb[:].rearrange("p (h e) -> p h e", h=4))

        for mu in range(4):       # q chunks 0,1; k chunks 2,3
            wqf = wst.tile([P, KC, P], F32, tag="wf")
            nc.sync.dma_start(wqf[:], w_qk_r.ap()[mu])
            wqb = wst.tile([P, KC, P], BF16, tag="wb")
            nc.scalar.activation(wqb[:], wqf[:], AF.Copy)
            for tau in range(4):
                ps = ps_acc.tile([P, TT], F32, tag="acc")
                for kc in range(KC):
                    nc.tensor.matmul(
                        ps[:], wqb[:, kc, :], hT_full[:, 8 * tau + kc, :],
                        start=(kc == 0), stop=(kc == KC - 1))
                dst = qT if mu < 2 else kT
                nc.vector.tensor_scalar_add(
                    dst[:, mu % 2, ts(tau, TT)], ps[:], b_qk_sb[:, mu:mu + 1])
        # ---------------- phase 4: attention
        ctxT = pers.tile([P, 2, N], BF16)
        for a in range(2):
            eb_sb = ebp.tile([P, 2, EB_J], BF16, tag="eb")
            nc.sync.dma_start(
                eb_sb[:], eb_in.ap()[2 * a:2 * a + 2].rearrange("h p j -> p h j"))
            for tau in range(4):
                cps0 = ps_ctx.tile([65, TT], F32, tag="ctx")
                cps1 = ps_ctx.tile([65, TT], F32, tag="ctx")
                cps = [cps0, cps1]
                for blk in range(NBLK):
                    col0 = EB_A - P * (blk - 4 * tau)
                    sps = []
                    for o in range(2):
                        sp = ps_acc.tile([P, TT], F32, tag="acc")
                        nc.tensor.matmul(
                            sp[:],
                            kT[64 * o:64 * o + 64, a, ts(blk, P)],
                            qT[64 * o:64 * o + 64, a, ts(tau, TT)],
                            start=True, stop=True)
                        sps.append(sp)
                    for o in range(2):
                        h = 2 * a + o
                        tsb = work.tile([P, TT], BF16, tag="wbf", bufs=6)
                        nc.scalar.activation(tsb[:], sps[o][:], AF.Exp, scale=0.125)
                        esb = work.tile([P, TT], BF16, tag="wbf", bufs=6)
                        nc.vector.tensor_tensor(
                            esb[:], tsb[:], eb_sb[:, o, col0:col0 + TT], op=OP.mult)
                        nc.tensor.matmul(
                            cps[o][:], v_aug[:, blk, 65 * h:65 * h + 65], esb[:],
                            start=(blk == 0), stop=(blk == NBLK - 1))
                for o in range(2):
                    recip = work.tile([1, TT], F32, tag="rowtmp", bufs=4)
                    nc.vector.reciprocal(recip[:], cps[o][64:65, :])
                    bc = ps_bc.tile([64, TT], F32, tag="bc")
                    nc.tensor.matmul(bc[:], ones_row[:, 0:64], recip[:],
                                     start=True, stop=True)
                    csb = work.tile([64, TT], BF16, tag="wbf", bufs=6)
                    nc.scalar.activation(csb[:], cps[o][0:64, :], AF.Copy)
                    nc.vector.tensor_tensor(
                        ctxT[64 * o:64 * o + 64, a, ts(tau, TT)],
                        csb[:], bc[:], op=OP.mult)

        # ---------------- phase 5: head-sharded out-proj partials + RS(add)
        # partial attn_out^T over own 4 heads (ctx dims 256), ALL tokens
        wof = wst.tile([P, 2, HID], F32, tag="wf")
        nc.sync.dma_start(wof[:], w_out_s.ap())
        wob = wst.tile([P, 2, HID], BF16, tag="wb")
        nc.vector.tensor_copy(wob[:], wof[:])
        po_sb = big.tile([P, KC, N], BF16, tag="slot32")
        for tau in range(4):
            for mu in range(KC):
                ps = ps_acc.tile([P, TT], F32, tag="acc")
                for kc in range(2):
                    nc.tensor.matmul(
                        ps[:], wob[:, kc, ts(mu, P)],
                        ctxT[:, kc, ts(tau, TT)],
                        start=(kc == 0), stop=(kc == 1))
                nc.vector.tensor_copy(po_sb[:, mu, ts(tau, TT)], ps[:])
        rs_bounce_in = dram.tile([4 * HID, TT], BF16)
        for j in range(4):
            nc.sync.dma_start(
                rs_bounce_in[:][ts(j, HID), :].rearrange("(c p) t -> p c t", p=P),
                po_sb[:, :, ts(j, TT)])
        rs_bounce_out = dram.tile([HID, TT], BF16)
        if sim:
            nc.sync.dma_start(rs_bounce_out[:], rs_bounce_in[:][0:HID, :])
        else:
            nc.gpsimd.collective_compute(
                "ReduceScatter", OP.add, replica_groups=RG4,
                ins=[rs_bounce_in.opt()], outs=[rs_bounce_out.opt()])
        ao_sb = pers.tile([P, KC, TT], BF16)
        nc.sync.dma_start(
            ao_sb[:], rs_bounce_out[:].rearrange("(c p) t -> p c t", p=P))

        # ---------------- phase 6: residual + LN2
        x2T = pers.tile([P, KC, TT], F32)
        for mu in range(KC):
            tmp = work.tile([P, TT], F32, tag="wf32", bufs=5)
            nc.vector.tensor_scalar(
                tmp[:], ao_sb[:, mu, :], b_out_sb[:, mu:mu + 1], mod_chunk(2, mu),
                op0=OP.add, op1=OP.mult)
            nc.vector.tensor_add(x2T[:, mu, :], tmp[:], xT[:, mu, :])

        m2_bc, r2_bc = ln_stats(x2T, "ln2")
        h2T = pers.tile([P, KC, TT], BF16)
        for kc in range(KC):
            t0 = work.tile([P, TT], F32, tag="wf32", bufs=5)
            nc.vector.tensor_sub(t0[:], x2T[:, kc, :], m2_bc[:])
            t1 = work.tile([P, TT], F32, tag="wf32", bufs=5)
            nc.vector.tensor_tensor(t1[:], t0[:], r2_bc[:], op=OP.mult)
            nc.vector.tensor_scalar(
                h2T[:, kc, :], t1[:], sc1p_mlp[:, kc:kc + 1], mod_chunk(3, kc),
                op0=OP.mult, op1=OP.add)

        # ---------------- phase 7: MLP (token-sharded, weights streamed)
        gT = big.tile([P, MLPH // P, TT], BF16, tag="slot32")
        for nu in range(MLPH // P):
            w1f = wst.tile([P, KC, P], F32, tag="wf")
            nc.sync.dma_start(w1f[:], w_mlp1.ap()[nu])
            w1b = wst.tile([P, KC, P], BF16, tag="wb")
            nc.scalar.activation(w1b[:], w1f[:], AF.Copy)
            ps = ps_acc.tile([P, TT], F32, tag="acc")
            for kc in range(KC):
                nc.tensor.matmul(ps[:], w1b[:, kc, :], h2T[:, kc, :],
                                 start=(kc == 0), stop=(kc == KC - 1))
            nc.scalar.activation(
                gT[:, nu, :], ps[:], AF.Gelu_apprx_tanh, bias=b_mlp1_sb[:, nu:nu + 1])
        for mu in range(KC):
            ps = ps_acc.tile([P, TT], F32, tag="acc")
            for half in range(2):
                w2f = wst.tile([P, 16, P], F32, tag="wf")
                nc.sync.dma_start(w2f[:], w_mlp2.ap()[mu, half])
                w2b = wst.tile([P, 16, P], BF16, tag="wb")
                nc.vector.tensor_copy(w2b[:], w2f[:])
                for kc in range(16):
                    gkc = 16 * half + kc
                    nc.tensor.matmul(ps[:], w2b[:, kc, :], gT[:, gkc, :],
                                     start=(gkc == 0), stop=(gkc == MLPH // P - 1))
            tmp = work.tile([P, TT], F32, tag="wf32", bufs=5)
            nc.vector.tensor_scalar(
                tmp[:], ps[:], b_mlp2_sb[:, mu:mu + 1], mod_chunk(5, mu),
                op0=OP.add, op1=OP.mult)
            outT = work.tile([P, TT], F32, tag="wf32", bufs=5)
            nc.vector.tensor_add(outT[:], tmp[:], x2T[:, mu, :])
            for r in range(TT // P):
                tps = ps_acc.tile([P, P], F32, tag="acc")
                nc.tensor.transpose(tps[:], outT[:, ts(r, P)], ident[:])
                osb = work.tile([P, P], F32, tag="osb", bufs=4)
                nc.vector.tensor_copy(osb[:], tps[:])
                nc.sync.dma_start(out_t.ap()[ts(r, P), ts(mu, P)], osb[:])

    nc.compile()
    return nc


# ---------------------------------------------------------------- runner



class SpmdRunner:
    def __init__(self, nc, n_cores):
        install_neuronx_cc_hook()
        self.nc = nc
        self.n_cores = n_cores
        partition_name = nc.partition_id_tensor.name if nc.partition_id_tensor else None
        in_names, out_names, out_avals = [], [], []
        for alloc in nc.m.functions[0].allocations:
            if not isinstance(alloc, mybir.MemoryLocationSet):
                continue
            name = alloc.memorylocations[0].name
            if alloc.kind == "ExternalInput":
                if name != partition_name:
                    in_names.append(name)
            elif alloc.kind == "ExternalOutput":
                out_names.append(name)
                out_avals.append(
                    jax.core.ShapedArray(tuple(alloc.tensor_shape), mybir.dt.np(alloc.dtype))
                )
        self.in_names, self.out_names, self.out_avals = in_names, out_names, out_avals
        n_params = len(in_names)
        n_outs = len(out_avals)
        all_in_names = list(in_names) + list(out_names)
        if partition_name is not None:
            all_in_names.append(partition_name)

        def _body(*args):
            operands = list(args)
            if partition_name is not None:
                operands.append(partition_id_tensor())
            return tuple(
                _bass_exec_p.bind(
                    *operands,
                    out_avals=tuple(out_avals),
                    in_names=tuple(all_in_names),
                    out_names=tuple(out_names),
                    lowering_input_output_aliases=(),
                    sim_require_finite=True,
                    sim_require_nnan=True,
                    nc=nc,
                )
            )

        devices = jax.devices()[:n_cores]
        self.mesh = Mesh(np.asarray(devices), ("core",))
        donate = tuple(range(n_params, n_params + n_outs))
        self.fn = jax.jit(
            shard_map(
                _body,
                mesh=self.mesh,
                in_specs=(PartitionSpec("core"),) * (n_params + n_outs),
                out_specs=(PartitionSpec("core"),) * n_outs,
                check_rep=False,
            ),
            donate_argnums=donate,
            keep_unused=True,
        )
        self.n_params, self.n_outs = n_params, n_outs

    def _concat_inputs(self, in_maps):
        return [
            np.concatenate([np.asarray(in_maps[c][n]) for c in range(self.n_cores)], axis=0)
            for n in self.in_names
        ]

    def run(self, in_maps):
        sharding = jax.sharding.NamedSharding(self.mesh, PartitionSpec("core"))
        concat_in = [
            jax.device_put(x, sharding) for x in self._concat_inputs(in_maps)
        ]
        zeros = [
            jax.device_put(
                np.zeros((self.n_cores * a.shape[0], *a.shape[1:]), a.dtype), sharding)
            for a in self.out_avals
        ]
        outs = self.fn(*concat_in, *zeros)
        return self._split(outs)

    def _split(self, out_arrs):
        return [
            {
                n: np.asarray(out_arrs[i]).reshape(self.n_cores, *self.out_avals[i].shape)[c]
                for i, n in enumerate(self.out_names)
            }
            for c in range(self.n_cores)
        ]

    def bench(self, in_maps, iters=30, warmup=3):
        """Chained repeated execution: output buffers of call i are donated as
        the output operands of call i+1, serializing calls on-device."""
        sharding = jax.sharding.NamedSharding(self.mesh, PartitionSpec("core"))
        concat_in = [jax.device_put(x, sharding) for x in self._concat_inputs(in_maps)]
        outs = tuple(
            jax.device_put(
                np.zeros((self.n_cores * a.shape[0], *a.shape[1:]), a.dtype), sharding)
            for a in self.out_avals
        )
        for _ in range(warmup):
            outs = self.fn(*concat_in, *outs)
        jax.block_until_ready(outs)
        t0 = time.perf_counter()
        for _ in range(iters):
            outs = self.fn(*concat_in, *outs)
        jax.block_until_ready(outs)
        t1 = time.perf_counter()
        return (t1 - t0) / iters, self._split(outs)


_CACHE = {}


def kernel(**inputs):
    """Full-input DiT block on 8 NeuronCores; returns full [B, N, HID] f32."""
    if "nc" not in _CACHE:
        _CACHE["nc"] = build_kernel()
        _CACHE["runner"] = SpmdRunner(_CACHE["nc"], 8)
    maps = make_in_maps(inputs)
    results = _CACHE["runner"].run(maps)
    return assemble_output(results)



# revision 4
# speedup vs baseline: 1.1235x; 1.1235x over previous
"""DiT block Bass kernel for 8 TRN2 NeuronCores.

Core i -> (b = i//4, g = i%4): batch item b; head group 4g..4g+3; token
quarter [512g, 512g+512) of batch b.  Activations are hidden-major
("transposed", [hidden_chunk=128, tokens]) throughout; the host pre-
transposes x per core and post-transposes the hidden-major output.
Collectives: AllGather(4) for mod + h, ReduceScatter(add) for attn out.
Weights ship as bf16 from the host; matmuls bf16 with f32 PSUM
accumulate; LN-stat / broadcast matmuls run in float32r (1 cycle/row).
Softmax is computed without max-subtraction (scores are provably small)
with the relative bias applied multiplicatively post-exp from a host-
precomputed diagonal-shifted exp(bias) table.
"""
import contextlib
import time
import numpy as np
import ml_dtypes
import jax
from jax.sharding import Mesh, PartitionSpec
from jax.experimental.shard_map import shard_map

import concourse.bass as bass
import concourse.mybir as mybir
import concourse.tile as tile
from concourse import bacc
from concourse.bass2jax import _bass_exec_p, install_neuronx_cc_hook, partition_id_tensor

F32 = mybir.dt.float32
F32R = mybir.dt.float32r
BF16 = mybir.dt.bfloat16
AF = mybir.ActivationFunctionType
OP = mybir.AluOpType
ts = bass.ts

B, N, HID = 2, 2048, 1024
NH, HD = 16, 64
MLPH = 4 * HID
NB, MAXD = 32, 128
P = 128
TT = 512
KC = HID // P          # 8
NBLK = N // P          # 16
EB_A = 1920
EB_J = 3968
RG4 = [[0, 1, 2, 3], [4, 5, 6, 7]]

BF = ml_dtypes.bfloat16


# ---------------------------------------------------------------- host prep
def rel_bucket_np(d):
    nb = NB // 2
    buckets = np.where(d > 0, nb, 0).astype(np.int64)
    rp = np.abs(d)
    max_exact = nb // 2
    is_small = rp < max_exact
    log_ratio = np.log(np.maximum(rp, 1).astype(np.float32) / np.float32(max_exact))
    rpl = max_exact + (
        log_ratio / np.float32(np.log(MAXD / max_exact)) * (nb - max_exact)
    ).astype(np.int32)
    rpl = np.minimum(rpl, nb - 1)
    return buckets + np.where(is_small, rp, rpl)


def make_eb_tables(rel_table):
    d = np.arange(-(N - 1), N)
    buck = rel_bucket_np(d)
    p = np.arange(P)[:, None]
    j = np.arange(EB_J)[None, :]
    dd = p + EB_A - j
    valid = (dd >= -(N - 1)) & (dd <= N - 1)
    idx = np.clip(dd + (N - 1), 0, 2 * N - 2)
    ebs = np.zeros((NH, P, EB_J), dtype=np.float32)
    for h in range(NH):
        bvec = rel_table[buck, h].astype(np.float32)
        tab = np.exp(bvec)[idx]
        tab[~valid] = 1.0
        ebs[h] = tab
    return ebs.astype(BF)


def make_in_maps(inputs):
    x = np.asarray(inputs["x"], np.float32)
    c = np.asarray(inputs["c"], np.float32)
    w_ada = np.asarray(inputs["w_ada"], np.float32)
    b_ada = np.asarray(inputs["b_ada"], np.float32)
    w_qkv = np.asarray(inputs["w_qkv"], np.float32)
    b_qkv = np.asarray(inputs["b_qkv"], np.float32)
    w_out = np.asarray(inputs["w_out"], np.float32)
    b_out = np.asarray(inputs["b_out"], np.float32)
    rel_table = np.asarray(inputs["rel_table"], np.float32)
    w_mlp1 = np.asarray(inputs["w_mlp1"], np.float32)
    b_mlp1 = np.asarray(inputs["b_mlp1"], np.float32)
    w_mlp2 = np.asarray(inputs["w_mlp2"], np.float32)
    b_mlp2 = np.asarray(inputs["b_mlp2"], np.float32)

    eb_all = make_eb_tables(rel_table)
    ones_col = np.ones((P, 1), np.float32)
    ones_row = np.ones((1, P), np.float32)

    maps = []
    for i in range(8):
        b, g = divmod(i, 4)
        qs, ks, vs = 256 * g, HID + 256 * g, 2 * HID + 256 * g
        w_qkv_s = np.concatenate(
            [w_qkv[:, qs:qs + 256], w_qkv[:, ks:ks + 256], w_qkv[:, vs:vs + 256]], 1)
        b_qk = np.concatenate([b_qkv[qs:qs + 256], b_qkv[ks:ks + 256]])
        bv = b_qkv[vs:vs + 256]
        maps.append({
            # x slice pre-transposed to hidden-major [P, KC, TT]
            "xT_own": np.ascontiguousarray(
                x[b, 512 * g:512 * (g + 1), :].T.reshape(KC, P, TT)
                .transpose(1, 0, 2)),
            "c_own": np.ascontiguousarray(c[b][:, None]),
            "w_ada_s": np.ascontiguousarray(
                w_ada[:, 1536 * g:1536 * (g + 1)].reshape(KC, P, 12, P)
                .transpose(2, 1, 0, 3).astype(BF)),
            "b_ada_s": np.ascontiguousarray(
                b_ada[1536 * g:1536 * (g + 1)].reshape(12, P).T),
            "w_qk_r": np.ascontiguousarray(
                w_qkv_s[:, :512].reshape(KC, P, 4, P).transpose(2, 1, 0, 3)
                .astype(BF)),
            "w_v_r": np.ascontiguousarray(
                w_qkv_s[:, 512:].reshape(KC, P, 256).transpose(1, 0, 2).astype(BF)),
            "b_qk_s": np.ascontiguousarray(b_qk.reshape(4, P).T),
            "b_v_bcast": np.ascontiguousarray(
                np.broadcast_to(bv[None, :], (P, 256)).astype(BF)),
            "w_out_s": np.ascontiguousarray(
                w_out[256 * g:256 * (g + 1), :].reshape(2, P, HID)
                .transpose(1, 0, 2).astype(BF)),
            "b_out_r": np.ascontiguousarray(b_out.reshape(KC, P).T),
            # nu-pair chunks [16, P, 2, KC, P]
            "w_mlp1": np.ascontiguousarray(
                w_mlp1.reshape(KC, P, 16, 2, P).transpose(2, 1, 3, 0, 4).astype(BF)),
            "b_mlp1_r": np.ascontiguousarray(b_mlp1.reshape(MLPH // P, P).T),
            # per-mu chunks [KC, P, 2, 16, P]
            "w_mlp2": np.ascontiguousarray(
                w_mlp2.reshape(2, 16, P, KC, P).transpose(3, 2, 0, 1, 4).astype(BF)),
            "b_mlp2_r": np.ascontiguousarray(b_mlp2.reshape(KC, P).T),
            "eb": np.ascontiguousarray(eb_all[4 * g:4 * g + 4]),
            "ones_col": ones_col,
            "ones_row": ones_row,
        })
    return maps


def assemble_output(results):
    out = np.zeros((B, N, HID), np.float32)
    for i in range(8):
        b, g = divmod(i, 4)
        # result is hidden-major [KC, P, TT]; transpose back to [TT, HID]
        r = np.asarray(results[i]["out"])
        out[b, 512 * g:512 * (g + 1), :] = r.transpose(2, 0, 1).reshape(TT, HID)
    return out


# ---------------------------------------------------------------- builder
def build_kernel(sim=False):
    nc = bacc.Bacc("TRN2", target_bir_lowering=False, debug=False, num_devices=8)

    din = lambda nm, sh, dt=F32: nc.dram_tensor(nm, sh, dt, kind="ExternalInput")
    xT_own = din("xT_own", [P, KC, TT])
    c_own = din("c_own", [HID, 1])
    w_ada_s = din("w_ada_s", [12, P, KC, P], BF16)
    b_ada_s = din("b_ada_s", [P, 12])
    w_qk_r = din("w_qk_r", [4, P, KC, P], BF16)
    w_v_r = din("w_v_r", [P, KC, 256], BF16)
    b_qk_s = din("b_qk_s", [P, 4])
    b_v_bcast = din("b_v_bcast", [P, 256], BF16)
    w_out_s = din("w_out_s", [P, 2, HID], BF16)
    b_out_r = din("b_out_r", [P, KC])
    w_mlp1 = din("w_mlp1", [16, P, 2, KC, P], BF16)
    b_mlp1_r = din("b_mlp1_r", [P, MLPH // P])
    w_mlp2 = din("w_mlp2", [KC, P, 2, 16, P], BF16)
    b_mlp2_r = din("b_mlp2_r", [P, KC])
    eb_in = din("eb", [4, P, EB_J], BF16)
    ones_col_in = din("ones_col", [P, 1])
    ones_row_in = din("ones_row", [1, P])

    out_t = nc.dram_tensor("out", [KC, P, TT], F32, kind="ExternalOutput")

    with tile.TileContext(nc) as tc, contextlib.ExitStack() as ctx:
        const = ctx.enter_context(tc.tile_pool(name="const", bufs=1))
        pers = ctx.enter_context(tc.tile_pool(name="pers", bufs=1))
        big = ctx.enter_context(tc.tile_pool(name="big", bufs=1))
        work = ctx.enter_context(tc.tile_pool(name="work", bufs=3))
        wst = ctx.enter_context(tc.tile_pool(name="wst", bufs=2))
        dram = ctx.enter_context(tc.tile_pool(name="dram", bufs=1, space="DRAM"))
        ebp = ctx.enter_context(tc.tile_pool(name="ebp", bufs=2))
        ps_acc = ctx.enter_context(tc.tile_pool(name="ps_acc", bufs=4, space="PSUM"))
        ps_bc = ctx.enter_context(tc.tile_pool(name="ps_bc", bufs=2, space="PSUM"))
        ps_ctx = ctx.enter_context(tc.tile_pool(name="ps_ctx", bufs=2, space="PSUM"))

        # ---------------- constants
        ones_col = const.tile([P, 1], F32)
        nc.sync.dma_start(ones_col[:], ones_col_in.ap())
        ones_row = const.tile([1, P], F32)
        nc.sync.dma_start(ones_row[:], ones_row_in.ap())
        b_qk_sb = const.tile([P, 4], F32)
        nc.sync.dma_start(b_qk_sb[:], b_qk_s.ap())
        b_v_sb = const.tile([P, 256], BF16)
        nc.sync.dma_start(b_v_sb[:], b_v_bcast.ap())
        b_out_sb = const.tile([P, KC], F32)
        nc.sync.dma_start(b_out_sb[:], b_out_r.ap())
        b_mlp1_sb = const.tile([P, MLPH // P], F32)
        nc.sync.dma_start(b_mlp1_sb[:], b_mlp1_r.ap())
        b_mlp2_sb = const.tile([P, KC], F32)
        nc.sync.dma_start(b_mlp2_sb[:], b_mlp2_r.ap())
        b_ada_sb = const.tile([P, 12], F32)
        nc.sync.dma_start(b_ada_sb[:], b_ada_s.ap())
        eps_sb = const.tile([1, 1], F32)
        nc.vector.memset(eps_sb[:], 1e-6)
        # bf16 constants for 1-cycle/row stat & broadcast matmuls
        ones_col_b = const.tile([P, 1], BF16)
        nc.vector.tensor_copy(ones_col_b[:], ones_col[:])
        ones_row_b = const.tile([1, P], BF16)
        nc.vector.tensor_copy(ones_row_b[:], ones_row[:])

        # ---------------- phase 1a: xT direct load (host pre-transposed)
        xT = pers.tile([P, KC, TT], F32)
        nc.sync.dma_start(xT[:], xT_own.ap())

        # ---------------- phase 0: mod shard (this core: w_ada cols 1536g..)
        cT_sb = pers.tile([P, KC], F32)
        nc.sync.dma_start(cT_sb[:], c_own.ap().rearrange("(c p) o -> p (c o)", p=P))
        silu_sb = pers.tile([P, KC], BF16)
        nc.scalar.activation(silu_sb[:], cT_sb[:], AF.Silu)
        mod_sh_sb = pers.tile([P, 12], F32)
        for mu in range(12):
            wa = wst.tile([P, KC, P], BF16, tag="wa")
            nc.sync.dma_start(wa[:], w_ada_s.ap()[mu])
            mps = ps_acc.tile([P, 1], F32, tag="acc")
            for kc in range(KC):
                nc.tensor.matmul(mps[:], wa[:, kc, :], silu_sb[:, kc:kc + 1],
                                 start=(kc == 0), stop=(kc == KC - 1))
            nc.vector.tensor_scalar_add(
                mod_sh_sb[:, mu:mu + 1], mps[:], b_ada_sb[:, mu:mu + 1])
        mod_bounce_in = dram.tile([P, 12], F32)
        nc.sync.dma_start(mod_bounce_in[:], mod_sh_sb[:])
        mod_bounce_out = dram.tile([4 * P, 12], F32)
        if sim:
            nc.sync.dma_start(mod_bounce_out[:][0:P, :], mod_bounce_in[:])
        else:
            nc.gpsimd.collective_compute(
                "AllGather", OP.bypass, replica_groups=RG4,
                ins=[mod_bounce_in.opt()], outs=[mod_bounce_out.opt()])
        mod_sb = pers.tile([P, 4, 12], F32)
        nc.sync.dma_start(
            mod_sb[:], mod_bounce_out[:].rearrange("(g p) j -> p g j", p=P))

        def mod_chunk(vec_idx, kc):
            gc = 8 * vec_idx + kc
            return mod_sb[:, gc // 12, gc % 12:gc % 12 + 1]

        sc1p_msa = pers.tile([P, KC], F32)
        sc1p_mlp = pers.tile([P, KC], F32)
        for kc in range(KC):
            nc.vector.tensor_scalar_add(sc1p_msa[:, kc:kc + 1], mod_chunk(1, kc), 1.0)
            nc.vector.tensor_scalar_add(sc1p_mlp[:, kc:kc + 1], mod_chunk(4, kc), 1.0)

        def ln_stats(src, tag):
            sum_ps = ps_acc.tile([1, TT], F32, tag="acc")
            for kc in range(KC):
                nc.tensor.matmul(sum_ps[:], ones_col[:], src[:, kc, :],
                                 start=(kc == 0), stop=(kc == KC - 1))
            sumsq_ps = ps_acc.tile([1, TT], F32, tag="acc")
            for kc in range(KC):
                sq = work.tile([P, TT], BF16, tag="wbf", bufs=6)
                nc.scalar.activation(sq[:], src[:, kc, :], AF.Square)
                nc.tensor.matmul(sumsq_ps[:], ones_col_b[:], sq[:],
                                 start=(kc == 0), stop=(kc == KC - 1))
            m_row = work.tile([1, TT], F32, tag="rowtmp", bufs=4)
            nc.vector.tensor_scalar_mul(m_row[:], sum_ps[:], 1.0 / HID)
            m_row_b = work.tile([1, TT], BF16, tag="rowtmpb", bufs=4)
            nc.vector.tensor_copy(m_row_b[:], m_row[:])
            msq = work.tile([1, TT], F32, tag="rowtmp", bufs=4)
            nc.vector.tensor_tensor(msq[:], m_row[:], m_row[:], op=OP.mult)
            var_row = work.tile([1, TT], F32, tag="rowtmp", bufs=4)
            nc.vector.scalar_tensor_tensor(
                var_row[:], sumsq_ps[:], 1.0 / HID, msq[:],
                op0=OP.mult, op1=OP.subtract)
            sd_row = work.tile([1, TT], F32, tag="rowtmp", bufs=4)
            nc.scalar.activation(sd_row[:], var_row[:], AF.Sqrt, bias=eps_sb[:])
            r_row = work.tile([1, TT], BF16, tag="rowtmpb", bufs=4)
            with nc.allow_low_precision(reason="rsqrt row in bf16 is ample"):
                nc.vector.reciprocal(r_row[:], sd_row[:])
            m_bc = ps_bc.tile([P, TT], F32, tag="bc")
            nc.tensor.matmul(m_bc[:], ones_row_b[:], m_row_b[:],
                             start=True, stop=True)
            r_bc = ps_bc.tile([P, TT], F32, tag="bc")
            nc.tensor.matmul(r_bc[:], ones_row_b[:], r_row[:],
                             start=True, stop=True)
            return m_bc, r_bc

        # ---------------- phase 2: hT own + AllGather
        m_bc, r_bc = ln_stats(xT, "ln1")
        hT_own = big.tile([P, KC, TT], BF16, tag="slot32")
        for kc in range(KC):
            t0 = work.tile([P, TT], F32, tag="wf32", bufs=5)
            nc.vector.tensor_sub(t0[:], xT[:, kc, :], m_bc[:])
            t1 = work.tile([P, TT], F32, tag="wf32", bufs=5)
            nc.vector.tensor_tensor(t1[:], t0[:], r_bc[:], op=OP.mult)
            nc.vector.tensor_scalar(
                hT_own[:, kc, :], t1[:], sc1p_msa[:, kc:kc + 1], mod_chunk(0, kc),
                op0=OP.mult, op1=OP.add)
        h_bounce_in_a = dram.tile([HID // 2, TT], BF16)
        h_bounce_in_b = dram.tile([HID // 2, TT], BF16)
        nc.sync.dma_start(
            h_bounce_in_a[:].rearrange("(c p) t -> p c t", p=P), hT_own[:, 0:4, :])
        nc.sync.dma_start(
            h_bounce_in_b[:].rearrange("(c p) t -> p c t", p=P), hT_own[:, 4:8, :])
        h_bounce_out_a = dram.tile([2 * HID, TT], BF16)
        h_bounce_out_b = dram.tile([2 * HID, TT], BF16)
        if sim:
            nc.sync.dma_start(h_bounce_out_a[:][0:HID // 2, :], h_bounce_in_a[:])
            nc.sync.dma_start(h_bounce_out_b[:][0:HID // 2, :], h_bounce_in_b[:])
        else:
            nc.gpsimd.collective_compute(
                "AllGather", OP.bypass, replica_groups=RG4,
                ins=[h_bounce_in_a.opt()], outs=[h_bounce_out_a.opt()])
            nc.gpsimd.collective_compute(
                "AllGather", OP.bypass, replica_groups=RG4,
                ins=[h_bounce_in_b.opt()], outs=[h_bounce_out_b.opt()])
        hT_full = big.tile([P, 32, TT], BF16, tag="slot32")
        for jq in range(4):
            nc.sync.dma_start(
                hT_full[:, KC * jq:KC * jq + 4, :],
                h_bounce_out_a[:][ts(jq, HID // 2), :].rearrange("(c p) t -> p c t", p=P))
            nc.sync.dma_start(
                hT_full[:, KC * jq + 4:KC * jq + 8, :],
                h_bounce_out_b[:][ts(jq, HID // 2), :].rearrange("(c p) t -> p c t", p=P))

        # ---------------- phase 3: qkv
        qT = pers.tile([P, 2, N], BF16)
        kT = pers.tile([P, 2, N], BF16)
        v_aug = pers.tile([P, NBLK, 260], BF16)
        nc.vector.memset(
            v_aug[:].rearrange("p b (h e) -> p b h e", h=4)[:, :, :, 64:65], 1.0)

        wvb = wst.tile([P, KC, 256], BF16, tag="wb")
        nc.sync.dma_start(wvb[:], w_v_r.ap())
        for blk in range(NBLK):
            ps = ps_acc.tile([P, 256], F32, tag="acc")
            for kc in range(KC):
                nc.tensor.matmul(
                    ps[:], hT_full[:, 8 * (blk // 4) + kc, ts(blk % 4, P)],
                    wvb[:, kc, :], start=(kc == 0), stop=(kc == KC - 1))
            vtmp = work.tile([P, 256], BF16, tag="wbf", bufs=6)
            nc.vector.tensor_copy(vtmp[:], ps[:])
            nc.vector.tensor_add(
                v_aug[:, blk, :].rearrange("p (h e) -> p h e", h=4)[:, :, 0:64],
                vtmp[:].rearrange("p (h e) -> p h e", h=4), b_v_sb[:].rearrange("p (h e) -> p h e", h=4))

        for mu in range(4):       # q chunks 0,1; k chunks 2,3
            wqb = wst.tile([P, KC, P], BF16, tag="wqb")
            nc.sync.dma_start(wqb[:], w_qk_r.ap()[mu])
            for tau in range(4):
                ps = ps_acc.tile([P, TT], F32, tag="acc")
                for kc in range(KC):
                    nc.tensor.matmul(
                        ps[:], wqb[:, kc, :], hT_full[:, 8 * tau + kc, :],
                        start=(kc == 0), stop=(kc == KC - 1))
                dst = qT if mu < 2 else kT
                nc.vector.tensor_scalar_add(
                    dst[:, mu % 2, ts(tau, TT)], ps[:], b_qk_sb[:, mu:mu + 1])
        # ---------------- phase 4: attention
        ctxT = pers.tile([P, 2, N], BF16)
        for a in range(2):
            eb_sb = ebp.tile([P, 2, EB_J], BF16, tag="eb")
            nc.sync.dma_start(
                eb_sb[:], eb_in.ap()[2 * a:2 * a + 2].rearrange("h p j -> p h j"))
            for tau in range(4):
                cps0 = ps_ctx.tile([65, TT], F32, tag="ctx")
                cps1 = ps_ctx.tile([65, TT], F32, tag="ctx")
                cps = [cps0, cps1]
                for blk in range(NBLK):
                    col0 = EB_A - P * (blk - 4 * tau)
                    sps = []
                    for o in range(2):
                        sp = ps_acc.tile([P, TT], F32, tag="acc")
                        nc.tensor.matmul(
                            sp[:],
                            kT[64 * o:64 * o + 64, a, ts(blk, P)],
                            qT[64 * o:64 * o + 64, a, ts(tau, TT)],
                            start=True, stop=True)
                        sps.append(sp)
                    for o in range(2):
                        h = 2 * a + o
                        tsb = work.tile([P, TT], BF16, tag="wbf", bufs=6)
                        nc.scalar.activation(tsb[:], sps[o][:], AF.Exp, scale=0.125)
                        esb = work.tile([P, TT], BF16, tag="wbf", bufs=6)
                        nc.vector.tensor_tensor(
                            esb[:], tsb[:], eb_sb[:, o, col0:col0 + TT], op=OP.mult)
                        nc.tensor.matmul(
                            cps[o][:], v_aug[:, blk, 65 * h:65 * h + 65], esb[:],
                            start=(blk == 0), stop=(blk == NBLK - 1))
                for o in range(2):
                    recip = work.tile([1, TT], BF16, tag="rowtmpb", bufs=4)
                    with nc.allow_low_precision(reason="softmax denom recip bf16"):
                        nc.vector.reciprocal(recip[:], cps[o][64:65, :])
                    bc = ps_bc.tile([64, TT], F32, tag="bc")
                    nc.tensor.matmul(bc[:], ones_row_b[:, 0:64], recip[:],
                                     start=True, stop=True)
                    csb = work.tile([64, TT], BF16, tag="wbf", bufs=6)
                    nc.scalar.activation(csb[:], cps[o][0:64, :], AF.Copy)
                    nc.vector.tensor_tensor(
                        ctxT[64 * o:64 * o + 64, a, ts(tau, TT)],
                        csb[:], bc[:], op=OP.mult)

        # ---------------- phase 5: head-sharded out-proj partials + RS(add)
        # partial attn_out^T over own 4 heads (ctx dims 256), ALL tokens
        wob = wst.tile([P, 2, HID], BF16, tag="wb")
        nc.sync.dma_start(wob[:], w_out_s.ap())
        po_sb = big.tile([P, KC, N], BF16, tag="slot32")
        for tau in range(4):
            for mu in range(KC):
                ps = ps_acc.tile([P, TT], F32, tag="acc")
                for kc in range(2):
                    nc.tensor.matmul(
                        ps[:], wob[:, kc, ts(mu, P)],
                        ctxT[:, kc, ts(tau, TT)],
                        start=(kc == 0), stop=(kc == 1))
                nc.vector.tensor_copy(po_sb[:, mu, ts(tau, TT)], ps[:])
        rs_bounce_in = dram.tile([4 * HID, TT], BF16)
        for j in range(4):
            nc.sync.dma_start(
                rs_bounce_in[:][ts(j, HID), :].rearrange("(c p) t -> p c t", p=P),
                po_sb[:, :, ts(j, TT)])
        rs_bounce_out = dram.tile([HID, TT], BF16)
        if sim:
            nc.sync.dma_start(rs_bounce_out[:], rs_bounce_in[:][0:HID, :])
        else:
            nc.gpsimd.collective_compute(
                "ReduceScatter", OP.add, replica_groups=RG4,
                ins=[rs_bounce_in.opt()], outs=[rs_bounce_out.opt()])
        ao_sb = pers.tile([P, KC, TT], BF16)
        nc.sync.dma_start(
            ao_sb[:], rs_bounce_out[:].rearrange("(c p) t -> p c t", p=P))

        # ---------------- phase 6: residual + LN2
        x2T = pers.tile([P, KC, TT], F32)
        for mu in range(KC):
            tmp = work.tile([P, TT], F32, tag="wf32", bufs=5)
            nc.vector.tensor_scalar(
                tmp[:], ao_sb[:, mu, :], b_out_sb[:, mu:mu + 1], mod_chunk(2, mu),
                op0=OP.add, op1=OP.mult)
            nc.vector.tensor_add(x2T[:, mu, :], tmp[:], xT[:, mu, :])

        m2_bc, r2_bc = ln_stats(x2T, "ln2")
        h2T = pers.tile([P, KC, TT], BF16)
        for kc in range(KC):
            t0 = work.tile([P, TT], F32, tag="wf32", bufs=5)
            nc.vector.tensor_sub(t0[:], x2T[:, kc, :], m2_bc[:])
            t1 = work.tile([P, TT], F32, tag="wf32", bufs=5)
            nc.vector.tensor_tensor(t1[:], t0[:], r2_bc[:], op=OP.mult)
            nc.vector.tensor_scalar(
                h2T[:, kc, :], t1[:], sc1p_mlp[:, kc:kc + 1], mod_chunk(3, kc),
                op0=OP.mult, op1=OP.add)

        # ---------------- phase 7: MLP (token-sharded, weights streamed bf16)
        gT = big.tile([P, MLPH // P, TT], BF16, tag="slot32")
        for nug in range(16):
            w1b = wst.tile([P, 2, KC, P], BF16, tag="wb")
            nc.sync.dma_start(w1b[:], w_mlp1.ap()[nug])
            for n in range(2):
                nu = 2 * nug + n
                ps = ps_acc.tile([P, TT], F32, tag="acc")
                for kc in range(KC):
                    nc.tensor.matmul(ps[:], w1b[:, n, kc, :], h2T[:, kc, :],
                                     start=(kc == 0), stop=(kc == KC - 1))
                nc.scalar.activation(
                    gT[:, nu, :], ps[:], AF.Gelu_apprx_tanh,
                    bias=b_mlp1_sb[:, nu:nu + 1])
        for mu in range(KC):
            w2b = wst.tile([P, 2, 16, P], BF16, tag="wb")
            nc.sync.dma_start(w2b[:], w_mlp2.ap()[mu])
            ps = ps_acc.tile([P, TT], F32, tag="acc")
            for half in range(2):
                for kc in range(16):
                    gkc = 16 * half + kc
                    nc.tensor.matmul(ps[:], w2b[:, half, kc, :], gT[:, gkc, :],
                                     start=(gkc == 0), stop=(gkc == MLPH // P - 1))
            tmp = work.tile([P, TT], F32, tag="wf32", bufs=5)
            nc.vector.tensor_scalar(
                tmp[:], ps[:], b_mlp2_sb[:, mu:mu + 1], mod_chunk(5, mu),
                op0=OP.add, op1=OP.mult)
            outT = work.tile([P, TT], F32, tag="wf32", bufs=5)
            nc.vector.tensor_add(outT[:], tmp[:], x2T[:, mu, :])
            nc.sync.dma_start(out_t.ap()[mu], outT[:])

    nc.compile()
    return nc


# ---------------------------------------------------------------- runner



class SpmdRunner:
    def __init__(self, nc, n_cores):
        install_neuronx_cc_hook()
        self.nc = nc
        self.n_cores = n_cores
        partition_name = nc.partition_id_tensor.name if nc.partition_id_tensor else None
        in_names, out_names, out_avals = [], [], []
        for alloc in nc.m.functions[0].allocations:
            if not isinstance(alloc, mybir.MemoryLocationSet):
                continue
            name = alloc.memorylocations[0].name
            if alloc.kind == "ExternalInput":
                if name != partition_name:
                    in_names.append(name)
            elif alloc.kind == "ExternalOutput":
                out_names.append(name)
                out_avals.append(
                    jax.core.ShapedArray(tuple(alloc.tensor_shape), mybir.dt.np(alloc.dtype))
                )
        self.in_names, self.out_names, self.out_avals = in_names, out_names, out_avals
        n_params = len(in_names)
        n_outs = len(out_avals)
        all_in_names = list(in_names) + list(out_names)
        if partition_name is not None:
            all_in_names.append(partition_name)

        def _body(*args):
            operands = list(args)
            if partition_name is not None:
                operands.append(partition_id_tensor())
            return tuple(
                _bass_exec_p.bind(
                    *operands,
                    out_avals=tuple(out_avals),
                    in_names=tuple(all_in_names),
                    out_names=tuple(out_names),
                    lowering_input_output_aliases=(),
                    sim_require_finite=True,
                    sim_require_nnan=True,
                    nc=nc,
                )
            )

        devices = jax.devices()[:n_cores]
        self.mesh = Mesh(np.asarray(devices), ("core",))
        donate = tuple(range(n_params, n_params + n_outs))
        self.fn = jax.jit(
            shard_map(
                _body,
                mesh=self.mesh,
                in_specs=(PartitionSpec("core"),) * (n_params + n_outs),
                out_specs=(PartitionSpec("core"),) * n_outs,
                check_rep=False,
            ),
            donate_argnums=donate,
            keep_unused=True,
        )
        self.n_params, self.n_outs = n_params, n_outs

    def _concat_inputs(self, in_maps):
        return [
            np.concatenate([np.asarray(in_maps[c][n]) for c in range(self.n_cores)], axis=0)
            for n in self.in_names
        ]

    def run(self, in_maps):
        sharding = jax.sharding.NamedSharding(self.mesh, PartitionSpec("core"))
        concat_in = [
            jax.device_put(x, sharding) for x in self._concat_inputs(in_maps)
        ]
        zeros = [
            jax.device_put(
                np.zeros((self.n_cores * a.shape[0], *a.shape[1:]), a.dtype), sharding)
            for a in self.out_avals
        ]
        outs = self.fn(*concat_in, *zeros)
        return self._split(outs)

    def _split(self, out_arrs):
        return [
            {
                n: np.asarray(out_arrs[i]).reshape(self.n_cores, *self.out_avals[i].shape)[c]
                for i, n in enumerate(self.out_names)
            }
            for c in range(self.n_cores)
        ]

    def bench(self, in_maps, iters=30, warmup=3):
        """Chained repeated execution: output buffers of call i are donated as
        the output operands of call i+1, serializing calls on-device."""
        sharding = jax.sharding.NamedSharding(self.mesh, PartitionSpec("core"))
        concat_in = [jax.device_put(x, sharding) for x in self._concat_inputs(in_maps)]
        outs = tuple(
            jax.device_put(
                np.zeros((self.n_cores * a.shape[0], *a.shape[1:]), a.dtype), sharding)
            for a in self.out_avals
        )
        for _ in range(warmup):
            outs = self.fn(*concat_in, *outs)
        jax.block_until_ready(outs)
        t0 = time.perf_counter()
        for _ in range(iters):
            outs = self.fn(*concat_in, *outs)
        jax.block_until_ready(outs)
        t1 = time.perf_counter()
        return (t1 - t0) / iters, self._split(outs)


_CACHE = {}


def kernel(**inputs):
    """Full-input DiT block on 8 NeuronCores; returns full [B, N, HID] f32."""
    if "nc" not in _CACHE:
        _CACHE["nc"] = build_kernel()
        _CACHE["runner"] = SpmdRunner(_CACHE["nc"], 8)
    maps = make_in_maps(inputs)
    results = _CACHE["runner"].run(maps)
    return assemble_output(results)


# revision 9
# speedup vs baseline: 1.2576x; 1.1194x over previous
"""DiT block Bass kernel for 8 TRN2 NeuronCores.

Core i -> (b = i//4, g = i%4): batch item b; head group 4g..4g+3; token
quarter [512g, 512g+512) of batch b.  Activations are hidden-major
("transposed", [hidden_chunk=128, tokens]) throughout; the host pre-
transposes x per core and post-transposes the hidden-major output.
Collectives: AllGather(4) for mod + h, ReduceScatter(add) for attn out.
Weights ship as bf16 from the host; matmuls bf16 with f32 PSUM
accumulate; LN-stat / broadcast matmuls run with bf16 moving operands.
Softmax is computed without max-subtraction (scores are provably small)
with the relative bias applied multiplicatively post-exp from a host-
precomputed diagonal-shifted exp(bias) table.
"""
import contextlib
import time
import numpy as np
import ml_dtypes
import jax
from jax.sharding import Mesh, PartitionSpec
from jax.experimental.shard_map import shard_map

import concourse.bass as bass
import concourse.mybir as mybir
import concourse.tile as tile
from concourse import bacc
from concourse.bass2jax import _bass_exec_p, install_neuronx_cc_hook, partition_id_tensor

F32 = mybir.dt.float32
F32R = mybir.dt.float32r
BF16 = mybir.dt.bfloat16
AF = mybir.ActivationFunctionType
OP = mybir.AluOpType
ts = bass.ts

B, N, HID = 2, 2048, 1024
NH, HD = 16, 64
MLPH = 4 * HID
NB, MAXD = 32, 128
P = 128
TT = 512
KC = HID // P          # 8
NBLK = N // P          # 16
EB_A = 1920
EB_J = 3968
BJ0 = 1408          # eb band start col (non-saturated |d|<128 region for blk-4tau in [-1,4])
BJW = 1152          # eb band width
RG4 = [[0, 1, 2, 3], [4, 5, 6, 7]]

BF = ml_dtypes.bfloat16


# ---------------------------------------------------------------- host prep
def rel_bucket_np(d):
    nb = NB // 2
    buckets = np.where(d > 0, nb, 0).astype(np.int64)
    rp = np.abs(d)
    max_exact = nb // 2
    is_small = rp < max_exact
    log_ratio = np.log(np.maximum(rp, 1).astype(np.float32) / np.float32(max_exact))
    rpl = max_exact + (
        log_ratio / np.float32(np.log(MAXD / max_exact)) * (nb - max_exact)
    ).astype(np.int32)
    rpl = np.minimum(rpl, nb - 1)
    return buckets + np.where(is_small, rp, rpl)


def make_eb_tables(rel_table):
    d = np.arange(-(N - 1), N)
    buck = rel_bucket_np(d)
    p = np.arange(P)[:, None]
    j = np.arange(EB_J)[None, :]
    dd = p + EB_A - j
    valid = (dd >= -(N - 1)) & (dd <= N - 1)
    idx = np.clip(dd + (N - 1), 0, 2 * N - 2)
    ebs = np.zeros((NH, P, EB_J), dtype=np.float32)
    for h in range(NH):
        bvec = rel_table[buck, h].astype(np.float32)
        tab = np.exp(bvec)[idx]
        tab[~valid] = 1.0
        ebs[h] = tab
    return ebs.astype(BF)


def make_in_maps(inputs):
    x = np.asarray(inputs["x"], np.float32)
    c = np.asarray(inputs["c"], np.float32)
    w_ada = np.asarray(inputs["w_ada"], np.float32)
    b_ada = np.asarray(inputs["b_ada"], np.float32)
    w_qkv = np.asarray(inputs["w_qkv"], np.float32)
    b_qkv = np.asarray(inputs["b_qkv"], np.float32)
    w_out = np.asarray(inputs["w_out"], np.float32)
    b_out = np.asarray(inputs["b_out"], np.float32)
    rel_table = np.asarray(inputs["rel_table"], np.float32)
    w_mlp1 = np.asarray(inputs["w_mlp1"], np.float32)
    b_mlp1 = np.asarray(inputs["b_mlp1"], np.float32)
    w_mlp2 = np.asarray(inputs["w_mlp2"], np.float32)
    b_mlp2 = np.asarray(inputs["b_mlp2"], np.float32)

    eb_all = make_eb_tables(rel_table)          # [NH, P, EB_J] bf16
    eb_band_all = np.ascontiguousarray(eb_all[:, :, BJ0:BJ0 + BJW])
    # saturated-bucket constants per head: d <= -128 -> bucket 15, d >= 128 -> bucket 31
    c_neg = np.exp(rel_table[NB // 2 - 1, :]).astype(np.float32)   # [NH]
    c_plus = np.exp(rel_table[NB - 1, :]).astype(np.float32)
    ones_col = np.ones((P, 1), np.float32)
    ones_row = np.ones((1, P), np.float32)

    maps = []
    for i in range(8):
        b, g = divmod(i, 4)
        qs, ks, vs = 256 * g, HID + 256 * g, 2 * HID + 256 * g
        w_qkv_s = np.concatenate(
            [w_qkv[:, qs:qs + 256], w_qkv[:, ks:ks + 256], w_qkv[:, vs:vs + 256]], 1)
        b_qk = np.concatenate([b_qkv[qs:qs + 256], b_qkv[ks:ks + 256]])
        bv = b_qkv[vs:vs + 256]
        maps.append({
            # x slice pre-transposed to hidden-major [P, KC, TT]
            "xT_own": np.ascontiguousarray(
                x[b, 512 * g:512 * (g + 1), :].T.reshape(KC, P, TT)
                .transpose(1, 0, 2)),
            "c_own": np.ascontiguousarray(c[b][:, None]),
            "w_ada_s": np.ascontiguousarray(
                w_ada[:, 1536 * g:1536 * (g + 1)].reshape(KC, P, 2, 6, P)
                .transpose(2, 1, 3, 0, 4).astype(BF)),
            "b_ada_s": np.ascontiguousarray(
                b_ada[1536 * g:1536 * (g + 1)].reshape(12, P).T),
            "w_qk_r": np.ascontiguousarray(
                w_qkv_s[:, :512].reshape(KC, P, 4, P).transpose(2, 1, 0, 3)
                .astype(BF)),
            "w_v_r": np.ascontiguousarray(
                w_qkv_s[:, 512:].reshape(KC, P, 256).transpose(1, 0, 2).astype(BF)),
            "b_qk_s": np.ascontiguousarray(b_qk.reshape(4, P).T),
            "b_v_bcast": np.ascontiguousarray(
                np.broadcast_to(bv[None, :], (P, 256)).astype(BF)),
            "w_out_s": np.ascontiguousarray(
                w_out[256 * g:256 * (g + 1), :].reshape(2, P, HID)
                .transpose(1, 0, 2).astype(BF)),
            "b_out_r": np.ascontiguousarray(b_out.reshape(KC, P).T),
            # nu-pair chunks [16, P, 2, KC, P]
            "w_mlp1": np.ascontiguousarray(
                w_mlp1.reshape(KC, P, 16, 2, P).transpose(2, 1, 3, 0, 4).astype(BF)),
            "b_mlp1_r": np.ascontiguousarray(b_mlp1.reshape(MLPH // P, P).T),
            # per-mu chunks [KC, P, 2, 16, P]
            "w_mlp2": np.ascontiguousarray(
                w_mlp2.reshape(2, 16, P, KC, P).transpose(3, 2, 0, 1, 4).astype(BF)),
            "b_mlp2_r": np.ascontiguousarray(b_mlp2.reshape(KC, P).T),
            "eb": np.ascontiguousarray(eb_band_all[4 * g:4 * g + 4]),
            # [P, 260] broadcast rows: value 65h+e = c for head h (all e incl the ones slot)
            "cbn": np.ascontiguousarray(np.broadcast_to(
                np.repeat(c_neg[4 * g:4 * g + 4], 65)[None, :], (P, 260)).astype(BF)),
            "cbp": np.ascontiguousarray(np.broadcast_to(
                np.repeat(c_plus[4 * g:4 * g + 4], 65)[None, :], (P, 260)).astype(BF)),
            "ones_col": ones_col,
            "ones_row": ones_row,
        })
    return maps


def assemble_output(results):
    out = np.zeros((B, N, HID), np.float32)
    for i in range(8):
        b, g = divmod(i, 4)
        # result is hidden-major [KC, P, TT]; transpose back to [TT, HID]
        r = np.asarray(results[i]["out"])
        out[b, 512 * g:512 * (g + 1), :] = r.transpose(2, 0, 1).reshape(TT, HID)
    return out


# ---------------------------------------------------------------- builder
def build_kernel(sim=False):
    nc = bacc.Bacc("TRN2", target_bir_lowering=False, debug=False, num_devices=8)

    din = lambda nm, sh, dt=F32: nc.dram_tensor(nm, sh, dt, kind="ExternalInput")
    xT_own = din("xT_own", [P, KC, TT])
    c_own = din("c_own", [HID, 1])
    w_ada_s = din("w_ada_s", [2, P, 6, KC, P], BF16)
    b_ada_s = din("b_ada_s", [P, 12])
    w_qk_r = din("w_qk_r", [4, P, KC, P], BF16)
    w_v_r = din("w_v_r", [P, KC, 256], BF16)
    b_qk_s = din("b_qk_s", [P, 4])
    b_v_bcast = din("b_v_bcast", [P, 256], BF16)
    w_out_s = din("w_out_s", [P, 2, HID], BF16)
    b_out_r = din("b_out_r", [P, KC])
    w_mlp1 = din("w_mlp1", [16, P, 2, KC, P], BF16)
    b_mlp1_r = din("b_mlp1_r", [P, MLPH // P])
    w_mlp2 = din("w_mlp2", [KC, P, 2, 16, P], BF16)
    b_mlp2_r = din("b_mlp2_r", [P, KC])
    eb_in = din("eb", [4, P, BJW], BF16)
    cbn_in = din("cbn", [P, 260], BF16)
    cbp_in = din("cbp", [P, 260], BF16)
    ones_col_in = din("ones_col", [P, 1])
    ones_row_in = din("ones_row", [1, P])

    out_t = nc.dram_tensor("out", [KC, P, TT], F32, kind="ExternalOutput")

    with tile.TileContext(nc) as tc, contextlib.ExitStack() as ctx:
        const = ctx.enter_context(tc.tile_pool(name="const", bufs=1))
        pers = ctx.enter_context(tc.tile_pool(name="pers", bufs=1))
        big = ctx.enter_context(tc.tile_pool(name="big", bufs=1))
        work = ctx.enter_context(tc.tile_pool(name="work", bufs=3))
        wst = ctx.enter_context(tc.tile_pool(name="wst", bufs=2))
        dram = ctx.enter_context(tc.tile_pool(name="dram", bufs=1, space="DRAM"))
        ebp = ctx.enter_context(tc.tile_pool(name="ebp", bufs=2))
        ps_bc = ctx.enter_context(tc.tile_pool(name="ps_bc", bufs=2, space="PSUM"))
        ps_ctx = ctx.enter_context(tc.tile_pool(name="ps_ctx", bufs=2, space="PSUM"))
        ps_acc_scope = contextlib.ExitStack()
        ps_acc = ps_acc_scope.enter_context(
            tc.tile_pool(name="ps_acc", bufs=4, space="PSUM"))

        # ---------------- phase 1a: xT direct load (host pre-transposed)
        xT = pers.tile([P, KC, TT], F32)
        nc.sync.dma_start(xT[:], xT_own.ap())
        cT_sb = pers.tile([P, KC], F32)
        nc.sync.dma_start(cT_sb[:], c_own.ap().rearrange("(c p) o -> p (c o)", p=P))
        # ---------------- constants
        ones_col = const.tile([P, 1], F32)
        nc.sync.dma_start(ones_col[:], ones_col_in.ap())
        ones_row = const.tile([1, P], F32)
        nc.sync.dma_start(ones_row[:], ones_row_in.ap())
        b_qk_sb = const.tile([P, 4], F32)
        nc.sync.dma_start(b_qk_sb[:], b_qk_s.ap())
        b_v_sb = const.tile([P, 256], BF16)
        nc.sync.dma_start(b_v_sb[:], b_v_bcast.ap())
        b_out_sb = const.tile([P, KC], F32)
        nc.sync.dma_start(b_out_sb[:], b_out_r.ap())
        b_mlp1_sb = const.tile([P, MLPH // P], F32)
        nc.sync.dma_start(b_mlp1_sb[:], b_mlp1_r.ap())
        b_mlp2_sb = const.tile([P, KC], F32)
        nc.sync.dma_start(b_mlp2_sb[:], b_mlp2_r.ap())
        b_ada_sb = const.tile([P, 12], F32)
        nc.sync.dma_start(b_ada_sb[:], b_ada_s.ap())
        eps_sb = const.tile([1, 1], F32)
        nc.vector.memset(eps_sb[:], 1e-6)
        cbn_sb = const.tile([P, 260], BF16)
        nc.sync.dma_start(cbn_sb[:], cbn_in.ap())
        cbp_sb = const.tile([P, 260], BF16)
        nc.sync.dma_start(cbp_sb[:], cbp_in.ap())
        # bf16 constants for 1-cycle/row stat & broadcast matmuls
        ones_col_b = const.tile([P, 1], BF16)
        nc.vector.tensor_copy(ones_col_b[:], ones_col[:])
        ones_row_b = const.tile([1, P], BF16)
        nc.vector.tensor_copy(ones_row_b[:], ones_row[:])


        def ln_stats(src, pool):
            sum_ps = pool.tile([1, TT], F32, tag="acc")
            sumsq_ps = pool.tile([1, TT], F32, tag="acc")
            for kc in range(KC):
                if src.dtype == BF16:
                    xb = src[:, kc, :]
                else:
                    xbt = work.tile([P, TT], BF16, tag="wbf", bufs=4)
                    nc.scalar.activation(xbt[:], src[:, kc, :], AF.Copy)
                    xb = xbt[:]
                nc.tensor.matmul(sum_ps[:], ones_col_b[:], xb,
                                 start=(kc == 0), stop=(kc == KC - 1))
                sqb = work.tile([P, TT], BF16, tag="wbf", bufs=4)
                nc.vector.tensor_tensor(sqb[:], xb, xb, op=OP.mult)
                nc.tensor.matmul(sumsq_ps[:], ones_col_b[:], sqb[:],
                                 start=(kc == 0), stop=(kc == KC - 1))
            m_row = work.tile([1, TT], F32, tag="rowtmp", bufs=3)
            nc.vector.tensor_scalar_mul(m_row[:], sum_ps[:], 1.0 / HID)
            m_row_b = work.tile([1, TT], BF16, tag="rowtmpb", bufs=4)
            nc.vector.tensor_copy(m_row_b[:], m_row[:])
            msq = work.tile([1, TT], F32, tag="rowtmp", bufs=3)
            nc.vector.tensor_tensor(msq[:], m_row[:], m_row[:], op=OP.mult)
            var_row = work.tile([1, TT], F32, tag="rowtmp", bufs=3)
            nc.vector.scalar_tensor_tensor(
                var_row[:], sumsq_ps[:], 1.0 / HID, msq[:],
                op0=OP.mult, op1=OP.subtract)
            sd_row = work.tile([1, TT], F32, tag="rowtmp", bufs=3)
            nc.scalar.activation(sd_row[:], var_row[:], AF.Sqrt, bias=eps_sb[:])
            r_row = work.tile([1, TT], BF16, tag="rowtmpb", bufs=4)
            with nc.allow_low_precision(reason="rsqrt row in bf16 is ample"):
                nc.vector.reciprocal(r_row[:], sd_row[:])
            m_bc = ps_bc.tile([P, TT], F32, tag="bc")
            nc.tensor.matmul(m_bc[:], ones_row_b[:], m_row_b[:],
                             start=True, stop=True)
            r_bc = ps_bc.tile([P, TT], F32, tag="bc")
            nc.tensor.matmul(r_bc[:], ones_row_b[:], r_row[:],
                             start=True, stop=True)
            return m_bc, r_bc

        # LN1 stats first: PE queue not blocked by mod's weight stream
        m_bc, r_bc = ln_stats(xT, ps_acc)

        # ---------------- phase 0: mod shard (this core: w_ada cols 1536g..)
        silu_sb = pers.tile([P, KC], BF16)
        nc.scalar.activation(silu_sb[:], cT_sb[:], AF.Silu)
        mod_sh_sb = pers.tile([P, 12], F32)
        for half in range(2):
            wa = wst.tile([P, 6, KC, P], BF16, tag="wa", bufs=2)
            nc.sync.dma_start(wa[:], w_ada_s.ap()[half])
            for m6 in range(6):
                mu = 6 * half + m6
                mps = ps_acc.tile([P, 1], F32, tag="acc")
                for kc in range(KC):
                    nc.tensor.matmul(mps[:], wa[:, m6, kc, :], silu_sb[:, kc:kc + 1],
                                     start=(kc == 0), stop=(kc == KC - 1))
                nc.vector.tensor_scalar_add(
                    mod_sh_sb[:, mu:mu + 1], mps[:], b_ada_sb[:, mu:mu + 1])
        mod_bounce_in = dram.tile([P, 12], F32)
        nc.sync.dma_start(mod_bounce_in[:], mod_sh_sb[:])
        mod_bounce_out = dram.tile([4 * P, 12], F32)
        if sim:
            nc.sync.dma_start(mod_bounce_out[:][0:P, :], mod_bounce_in[:])
        else:
            nc.gpsimd.collective_compute(
                "AllGather", OP.bypass, replica_groups=RG4,
                ins=[mod_bounce_in.opt()], outs=[mod_bounce_out.opt()])
        mod_sb = pers.tile([P, 4, 12], F32)
        nc.sync.dma_start(
            mod_sb[:], mod_bounce_out[:].rearrange("(g p) j -> p g j", p=P))

        def mod_chunk(vec_idx, kc):
            gc = 8 * vec_idx + kc
            return mod_sb[:, gc // 12, gc % 12:gc % 12 + 1]

        sc1p_msa = pers.tile([P, KC], F32)
        sc1p_mlp = pers.tile([P, KC], F32)
        for kc in range(KC):
            nc.vector.tensor_scalar_add(sc1p_msa[:, kc:kc + 1], mod_chunk(1, kc), 1.0)
            nc.vector.tensor_scalar_add(sc1p_mlp[:, kc:kc + 1], mod_chunk(4, kc), 1.0)

        # ---------------- phase 2: hT own + AllGather
        hT_own = big.tile([P, KC, TT], BF16, tag="slot32")
        for kc in range(KC):
            t0 = work.tile([P, TT], F32, tag="wf32", bufs=4)
            nc.vector.tensor_sub(t0[:], xT[:, kc, :], m_bc[:])
            t1 = work.tile([P, TT], F32, tag="wf32", bufs=4)
            nc.vector.tensor_tensor(t1[:], t0[:], r_bc[:], op=OP.mult)
            nc.vector.tensor_scalar(
                hT_own[:, kc, :], t1[:], sc1p_msa[:, kc:kc + 1], mod_chunk(0, kc),
                op0=OP.mult, op1=OP.add)
        h_bounce_in_a = dram.tile([HID // 2, TT], BF16)
        h_bounce_in_b = dram.tile([HID // 2, TT], BF16)
        nc.sync.dma_start(
            h_bounce_in_a[:].rearrange("(c p) t -> p c t", p=P), hT_own[:, 0:4, :])
        nc.sync.dma_start(
            h_bounce_in_b[:].rearrange("(c p) t -> p c t", p=P), hT_own[:, 4:8, :])
        h_bounce_out_a = dram.tile([2 * HID, TT], BF16)
        h_bounce_out_b = dram.tile([2 * HID, TT], BF16)
        if sim:
            nc.sync.dma_start(h_bounce_out_a[:][0:HID // 2, :], h_bounce_in_a[:])
            nc.sync.dma_start(h_bounce_out_b[:][0:HID // 2, :], h_bounce_in_b[:])
        else:
            nc.gpsimd.collective_compute(
                "AllGather", OP.bypass, replica_groups=RG4,
                ins=[h_bounce_in_a.opt()], outs=[h_bounce_out_a.opt()])
            nc.gpsimd.collective_compute(
                "AllGather", OP.bypass, replica_groups=RG4,
                ins=[h_bounce_in_b.opt()], outs=[h_bounce_out_b.opt()])
        hT_full = big.tile([P, 32, TT], BF16, tag="slot32")
        for jq in range(4):
            nc.sync.dma_start(
                hT_full[:, KC * jq:KC * jq + 4, :],
                h_bounce_out_a[:][ts(jq, HID // 2), :].rearrange("(c p) t -> p c t", p=P))
            nc.sync.dma_start(
                hT_full[:, KC * jq + 4:KC * jq + 8, :],
                h_bounce_out_b[:][ts(jq, HID // 2), :].rearrange("(c p) t -> p c t", p=P))

        # ---------------- phase 3: qkv
        qT = pers.tile([P, 2, N], BF16)
        kT = pers.tile([P, 2, N], BF16)
        v_aug = pers.tile([P, NBLK, 260], BF16)
        v_aug_n = pers.tile([P, NBLK, 260], BF16)
        v_aug_p = pers.tile([P, NBLK, 260], BF16)
        nc.vector.memset(
            v_aug[:].rearrange("p b (h e) -> p b h e", h=4)[:, :, :, 64:65], 1.0)

        for mu in range(4):       # q chunks 0,1; k chunks 2,3
            wqb = wst.tile([P, KC, P], BF16, tag="wqb")
            nc.sync.dma_start(wqb[:], w_qk_r.ap()[mu])
            for tau in range(4):
                ps = ps_acc.tile([P, TT], F32, tag="acc")
                for kc in range(KC):
                    nc.tensor.matmul(
                        ps[:], wqb[:, kc, :], hT_full[:, 8 * tau + kc, :],
                        start=(kc == 0), stop=(kc == KC - 1))
                dst = qT if mu < 2 else kT
                nc.vector.tensor_scalar_add(
                    dst[:, mu % 2, ts(tau, TT)], ps[:], b_qk_sb[:, mu:mu + 1])
        wvb = wst.tile([P, KC, 256], BF16, tag="wb")
        nc.sync.dma_start(wvb[:], w_v_r.ap())
        for blk in range(NBLK):
            ps = ps_acc.tile([P, 256], F32, tag="acc")
            for kc in range(KC):
                nc.tensor.matmul(
                    ps[:], hT_full[:, 8 * (blk // 4) + kc, ts(blk % 4, P)],
                    wvb[:, kc, :], start=(kc == 0), stop=(kc == KC - 1))
            nc.vector.tensor_add(
                v_aug[:, blk, :].rearrange("p (h e) -> p h e", h=4)[:, :, 0:64],
                ps[:].rearrange("p (h e) -> p h e", h=4),
                b_v_sb[:].rearrange("p (h e) -> p h e", h=4))
            nc.vector.tensor_tensor(
                v_aug_n[:, blk, :], v_aug[:, blk, :], cbn_sb[:], op=OP.mult)
            nc.vector.tensor_tensor(
                v_aug_p[:, blk, :], v_aug[:, blk, :], cbp_sb[:], op=OP.mult)

        # ---------------- phase 4: attention (wide exp over head pairs)
        ps_acc_scope.close()
        ctxT = pers.tile([P, 2, N], BF16)
        with tc.tile_pool(name="ps_wide", bufs=2, space="PSUM") as ps_wide:
            for a in range(2):
                eb_sb = ebp.tile([P, 2, BJW], BF16, tag="eb")
                nc.sync.dma_start(
                    eb_sb[:], eb_in.ap()[2 * a:2 * a + 2].rearrange("h p j -> p h j"))
                for tau in range(4):
                    cps0 = ps_ctx.tile([65, TT], F32, tag="ctx")
                    cps1 = ps_ctx.tile([65, TT], F32, tag="ctx")
                    cps = [cps0, cps1]
                    for blk in range(NBLK):
                        delta = blk - 4 * tau
                        spw = ps_wide.tile([P, 2 * TT], F32, tag="spw")
                        for o in range(2):
                            nc.tensor.matmul(
                                spw[:, ts(o, TT)],
                                kT[64 * o:64 * o + 64, a, ts(blk, P)],
                                qT[64 * o:64 * o + 64, a, ts(tau, TT)],
                                start=True, stop=True)
                        esb = work.tile([P, 2 * TT], BF16, tag="wbf2", bufs=4)
                        nc.scalar.activation(esb[:], spw[:], AF.Exp, scale=0.125)
                        if -1 <= delta <= 4:
                            # near-diagonal: multiply the non-saturated bias band
                            col0 = EB_A - P * delta - BJ0
                            emb = work.tile([P, 2, TT], BF16, tag="wbf2", bufs=4)
                            nc.vector.tensor_tensor(
                                emb[:], esb[:].rearrange("p (o t) -> p o t", o=2),
                                eb_sb[:, :, col0:col0 + TT], op=OP.mult)
                            vsrc, mov = v_aug, (emb[:, 0, :], emb[:, 1, :])
                        else:
                            # saturated bucket: constant bias folded into scaled V
                            vsrc = v_aug_n if delta < 0 else v_aug_p
                            mov = (esb[:, 0:TT], esb[:, TT:2 * TT])
                        for o in range(2):
                            h = 2 * a + o
                            nc.tensor.matmul(
                                cps[o][:], vsrc[:, blk, 65 * h:65 * h + 65],
                                mov[o],
                                start=(blk == 0), stop=(blk == NBLK - 1))
                    for o in range(2):
                        recip = work.tile([1, TT], BF16, tag="rowtmpb", bufs=4)
                        with nc.allow_low_precision(reason="softmax denom recip bf16"):
                            nc.vector.reciprocal(recip[:], cps[o][64:65, :])
                        bc = ps_bc.tile([64, TT], F32, tag="bc")
                        nc.tensor.matmul(bc[:], ones_row_b[:, 0:64], recip[:],
                                         start=True, stop=True)
                        csb = work.tile([64, TT], BF16, tag="wbf", bufs=4)
                        nc.vector.tensor_copy(csb[:], cps[o][0:64, :])
                        nc.vector.tensor_tensor(
                            ctxT[64 * o:64 * o + 64, a, ts(tau, TT)],
                            csb[:], bc[:], op=OP.mult)

        ps_acc2_scope = contextlib.ExitStack()
        ps_acc2 = ps_acc2_scope.enter_context(
            tc.tile_pool(name="ps_acc2", bufs=4, space="PSUM"))

        # ---------------- phase 5: head-sharded out-proj partials + RS(add)
        # partial attn_out^T over own 4 heads (ctx dims 256), ALL tokens
        wob = wst.tile([P, 2, HID], BF16, tag="wb")
        nc.sync.dma_start(wob[:], w_out_s.ap())
        po_sb = big.tile([P, KC, N], BF16, tag="slot32")
        rs_bounce_in = dram.tile([4 * HID, TT], BF16)
        for tau in range(4):
            for mu in range(KC):
                ps = ps_acc2.tile([P, TT], F32, tag="acc")
                for kc in range(2):
                    nc.tensor.matmul(
                        ps[:], wob[:, kc, ts(mu, P)],
                        ctxT[:, kc, ts(tau, TT)],
                        start=(kc == 0), stop=(kc == 1))
                if mu % 2 == 0:
                    nc.vector.tensor_copy(po_sb[:, mu, ts(tau, TT)], ps[:])
                else:
                    nc.scalar.copy(po_sb[:, mu, ts(tau, TT)], ps[:])
            nc.sync.dma_start(
                rs_bounce_in[:][ts(tau, HID), :].rearrange("(c p) t -> p c t", p=P),
                po_sb[:, :, ts(tau, TT)])
        rs_bounce_out = dram.tile([HID, TT], BF16)
        if sim:
            nc.sync.dma_start(rs_bounce_out[:], rs_bounce_in[:][0:HID, :])
        else:
            nc.gpsimd.collective_compute(
                "ReduceScatter", OP.add, replica_groups=RG4,
                ins=[rs_bounce_in.opt()], outs=[rs_bounce_out.opt()])
        ao_sb = pers.tile([P, KC, TT], BF16)
        nc.sync.dma_start(
            ao_sb[:], rs_bounce_out[:].rearrange("(c p) t -> p c t", p=P))

        # ---------------- phase 6: residual + LN2
        x2T = pers.tile([P, KC, TT], BF16)
        for mu in range(KC):
            tmp = work.tile([P, TT], F32, tag="wf32", bufs=4)
            nc.vector.tensor_scalar(
                tmp[:], ao_sb[:, mu, :], b_out_sb[:, mu:mu + 1], mod_chunk(2, mu),
                op0=OP.add, op1=OP.mult)
            nc.vector.tensor_add(x2T[:, mu, :], tmp[:], xT[:, mu, :])

        m2_bc, r2_bc = ln_stats(x2T, ps_acc2)
        h2T = pers.tile([P, KC, TT], BF16)
        for kc in range(KC):
            t0 = work.tile([P, TT], F32, tag="wf32", bufs=4)
            nc.vector.tensor_sub(t0[:], x2T[:, kc, :], m2_bc[:])
            t1 = work.tile([P, TT], F32, tag="wf32", bufs=4)
            nc.vector.tensor_tensor(t1[:], t0[:], r2_bc[:], op=OP.mult)
            nc.vector.tensor_scalar(
                h2T[:, kc, :], t1[:], sc1p_mlp[:, kc:kc + 1], mod_chunk(3, kc),
                op0=OP.mult, op1=OP.add)

        # ---------------- phase 7: MLP (token-sharded, weights streamed bf16)
        gT = big.tile([P, MLPH // P, TT], BF16, tag="slot32")
        for nug in range(16):
            w1b = wst.tile([P, 2, KC, P], BF16, tag="wb")
            nc.sync.dma_start(w1b[:], w_mlp1.ap()[nug])
            for n in range(2):
                nu = 2 * nug + n
                ps = ps_acc2.tile([P, TT], F32, tag="acc")
                for kc in range(KC):
                    nc.tensor.matmul(ps[:], w1b[:, n, kc, :], h2T[:, kc, :],
                                     start=(kc == 0), stop=(kc == KC - 1))
                nc.scalar.activation(
                    gT[:, nu, :], ps[:], AF.Gelu_apprx_tanh,
                    bias=b_mlp1_sb[:, nu:nu + 1])
        for mu in range(KC):
            w2b = wst.tile([P, 2, 16, P], BF16, tag="wb")
            nc.sync.dma_start(w2b[:], w_mlp2.ap()[mu])
            ps = ps_acc2.tile([P, TT], F32, tag="acc")
            for half in range(2):
                for kc in range(16):
                    gkc = 16 * half + kc
                    nc.tensor.matmul(ps[:], w2b[:, half, kc, :], gT[:, gkc, :],
                                     start=(gkc == 0), stop=(gkc == MLPH // P - 1))
            tmp = work.tile([P, TT], F32, tag="wf32", bufs=4)
            nc.vector.tensor_scalar(
                tmp[:], ps[:], b_mlp2_sb[:, mu:mu + 1], mod_chunk(5, mu),
                op0=OP.add, op1=OP.mult)
            outT = work.tile([P, TT], F32, tag="wf32", bufs=4)
            nc.vector.tensor_add(outT[:], tmp[:], x2T[:, mu, :])
            nc.sync.dma_start(out_t.ap()[mu], outT[:])
        ps_acc2_scope.close()

    nc.compile()
    return nc


# ---------------------------------------------------------------- runner



class SpmdRunner:
    def __init__(self, nc, n_cores):
        install_neuronx_cc_hook()
        self.nc = nc
        self.n_cores = n_cores
        partition_name = nc.partition_id_tensor.name if nc.partition_id_tensor else None
        in_names, out_names, out_avals = [], [], []
        for alloc in nc.m.functions[0].allocations:
            if not isinstance(alloc, mybir.MemoryLocationSet):
                continue
            name = alloc.memorylocations[0].name
            if alloc.kind == "ExternalInput":
                if name != partition_name:
                    in_names.append(name)
            elif alloc.kind == "ExternalOutput":
                out_names.append(name)
                out_avals.append(
                    jax.core.ShapedArray(tuple(alloc.tensor_shape), mybir.dt.np(alloc.dtype))
                )
        self.in_names, self.out_names, self.out_avals = in_names, out_names, out_avals
        n_params = len(in_names)
        n_outs = len(out_avals)
        all_in_names = list(in_names) + list(out_names)
        if partition_name is not None:
            all_in_names.append(partition_name)

        def _body(*args):
            operands = list(args)
            if partition_name is not None:
                operands.append(partition_id_tensor())
            return tuple(
                _bass_exec_p.bind(
                    *operands,
                    out_avals=tuple(out_avals),
                    in_names=tuple(all_in_names),
                    out_names=tuple(out_names),
                    lowering_input_output_aliases=(),
                    sim_require_finite=True,
                    sim_require_nnan=True,
                    nc=nc,
                )
            )

        devices = jax.devices()[:n_cores]
        self.mesh = Mesh(np.asarray(devices), ("core",))
        donate = tuple(range(n_params, n_params + n_outs))
        self.fn = jax.jit(
            shard_map(
                _body,
                mesh=self.mesh,
                in_specs=(PartitionSpec("core"),) * (n_params + n_outs),
                out_specs=(PartitionSpec("core"),) * n_outs,
                check_rep=False,
            ),
            donate_argnums=donate,
            keep_unused=True,
        )
        self.n_params, self.n_outs = n_params, n_outs

    def _concat_inputs(self, in_maps):
        return [
            np.concatenate([np.asarray(in_maps[c][n]) for c in range(self.n_cores)], axis=0)
            for n in self.in_names
        ]

    def run(self, in_maps):
        sharding = jax.sharding.NamedSharding(self.mesh, PartitionSpec("core"))
        concat_in = [
            jax.device_put(x, sharding) for x in self._concat_inputs(in_maps)
        ]
        zeros = [
            jax.device_put(
                np.zeros((self.n_cores * a.shape[0], *a.shape[1:]), a.dtype), sharding)
            for a in self.out_avals
        ]
        outs = self.fn(*concat_in, *zeros)
        return self._split(outs)

    def _split(self, out_arrs):
        return [
            {
                n: np.asarray(out_arrs[i]).reshape(self.n_cores, *self.out_avals[i].shape)[c]
                for i, n in enumerate(self.out_names)
            }
            for c in range(self.n_cores)
        ]

    def bench(self, in_maps, iters=30, warmup=3):
        """Chained repeated execution: output buffers of call i are donated as
        the output operands of call i+1, serializing calls on-device."""
        sharding = jax.sharding.NamedSharding(self.mesh, PartitionSpec("core"))
        concat_in = [jax.device_put(x, sharding) for x in self._concat_inputs(in_maps)]
        outs = tuple(
            jax.device_put(
                np.zeros((self.n_cores * a.shape[0], *a.shape[1:]), a.dtype), sharding)
            for a in self.out_avals
        )
        for _ in range(warmup):
            outs = self.fn(*concat_in, *outs)
        jax.block_until_ready(outs)
        t0 = time.perf_counter()
        for _ in range(iters):
            outs = self.fn(*concat_in, *outs)
        jax.block_until_ready(outs)
        t1 = time.perf_counter()
        return (t1 - t0) / iters, self._split(outs)


_CACHE = {}


def kernel(**inputs):
    """Full-input DiT block on 8 NeuronCores; returns full [B, N, HID] f32."""
    if "nc" not in _CACHE:
        _CACHE["nc"] = build_kernel()
        _CACHE["runner"] = SpmdRunner(_CACHE["nc"], 8)
    maps = make_in_maps(inputs)
    results = _CACHE["runner"].run(maps)
    return assemble_output(results)
